# revision 3
# baseline (speedup 1.0000x reference)
"""MultiHeadAttention (RoPE, causal) Trainium2 kernel over 8 NeuronCores.

Sharding: batch (2) x head-groups (4 heads each) -> 8 cores.
Each core computes, for its batch b and 4 heads:
  Q^T,K^T = (Wq/Wk chunk)^T @ x^T   (RoPE applied on-chip)
  S^T tiles = K^T_tile contract-d Q^T, exp (no max-sub; scores ~N(0,1)),
  causal mask via precomputed 0/1 tiles,
  O^T = V contract-k P^T; row-sums l via ones-matmul into psum row 0;
  1/l via ACT Ln -> broadcast matmul -> ACT Exp(scale=-1)  (one table set);
  O^T normalized on the PSUM->SBUF path, then Z_partial = O @ Wo_chunk.
Host sums the 4 per-core partials of each batch.

All matmuls run in float32r (full-rate PE); storage fp32.
"""

import sys

if "/opt/trn_rl_repo" not in sys.path:
    sys.path.insert(0, "/opt/trn_rl_repo")

import numpy as np

EMBED = 2048
S = 2048
NH = 16
HD = 128
B = 2
N_CORES = 8
HPC = 4              # heads per core
CW = HPC * HD        # 512: per-core projection width
SBK = 512            # s block width
NSB = S // SBK       # 4
NEC = EMBED // 128   # 16 e-chunks
NST = S // 128       # 16 s tiles / q tiles / k tiles
ROPE_BASE = 10000.0
SCALE = 1.0 / float(np.sqrt(HD))

_CACHE = {}


def _build_program():
    import concourse.bacc as bacc
    import concourse.mybir as mybir
    import concourse.tile as tile

    f32 = mybir.dt.float32
    f32r = mybir.dt.float32r
    EXP = mybir.ActivationFunctionType.Exp
    LN = mybir.ActivationFunctionType.Ln

    nc = bacc.Bacc("TRN2", target_bir_lowering=False, debug=False,
                   num_devices=N_CORES)

    xt_d = nc.dram_tensor("xt", [EMBED, S], f32, kind="ExternalInput").ap()
    wq_d = nc.dram_tensor("wq", [EMBED, CW], f32, kind="ExternalInput").ap()
    wk_d = nc.dram_tensor("wk", [EMBED, CW], f32, kind="ExternalInput").ap()
    wv_d = nc.dram_tensor("wv", [EMBED, CW], f32, kind="ExternalInput").ap()
    wo_d = nc.dram_tensor("wo", [CW, EMBED], f32, kind="ExternalInput").ap()
    cos_d = nc.dram_tensor("cost", [HD, S], f32, kind="ExternalInput").ap()
    sin_d = nc.dram_tensor("sints", [HD, S], f32, kind="ExternalInput").ap()
    msk_d = nc.dram_tensor("masks", [128, 4 * SBK], f32, kind="ExternalInput").ap()
    z_d = nc.dram_tensor("z", [S, EMBED], f32, kind="ExternalOutput").ap()

    with tile.TileContext(nc) as tc, \
         nc.allow_low_precision(reason="fp32r attention pipeline"):
        with tc.tile_pool(name="persist", bufs=1) as pp, \
             tc.tile_pool(name="ps", bufs=8, space="PSUM") as ps:
            qt = pp.tile([128, HPC * S], f32r, tag="qt")   # Q^T rope, per head
            kt = pp.tile([128, HPC * S], f32r, tag="kt")   # K^T rope, per head
            vt = pp.tile([128, NST * CW], f32r, tag="vt")  # V, [s-tile, 4 heads]

            # ---------------- Phase A: Q/K projections + RoPE ----------------
            with tc.tile_pool(name="wqk", bufs=1) as wp, \
                 tc.tile_pool(name="cossin", bufs=2) as cs, \
                 tc.tile_pool(name="xa", bufs=3) as xa, \
                 tc.tile_pool(name="ropetmp", bufs=1) as rp:
                wq_sb = wp.tile([128, NEC * CW], f32r, tag="wq")
                wk_sb = wp.tile([128, NEC * CW], f32r, tag="wk")

                for sb in range(NSB):
                    cos_sb = cs.tile([128, SBK], f32, tag="cos")
                    sin_sb = cs.tile([128, SBK], f32, tag="sin")
                    ss = slice(sb * SBK, (sb + 1) * SBK)
                    nc.sync.dma_start(cos_sb[:], cos_d[:, ss])
                    nc.sync.dma_start(sin_sb[:], sin_d[:, ss])
                    qp = [ps.tile([128, SBK], f32, tag="ps", name=f"qp{sb}_{_h}")
                          for _h in range(HPC)]
                    kp = [ps.tile([128, SBK], f32, tag="ps", name=f"kp{sb}_{_h}")
                          for _h in range(HPC)]
                    for ec in range(NEC):
                        xtile = xa.tile([128, SBK], f32r, tag="x")
                        nc.sync.dma_start(
                            xtile[:],
                            xt_d[ec * 128:(ec + 1) * 128,
                                 sb * SBK:(sb + 1) * SBK].bitcast(f32r))
                        if sb == 0:
                            # interleave weight loads with first s-block
                            nc.sync.dma_start(
                                wq_sb[:, ec * CW:(ec + 1) * CW],
                                wq_d[ec * 128:(ec + 1) * 128, :].bitcast(f32r))
                            nc.sync.dma_start(
                                wk_sb[:, ec * CW:(ec + 1) * CW],
                                wk_d[ec * 128:(ec + 1) * 128, :].bitcast(f32r))
                        st, sp = (ec == 0), (ec == NEC - 1)
                        for h in range(HPC):
                            wslice = slice(ec * CW + h * HD, ec * CW + (h + 1) * HD)
                            nc.tensor.matmul(qp[h][:], lhsT=wq_sb[:, wslice],
                                             rhs=xtile[:], start=st, stop=sp)
                            nc.tensor.matmul(kp[h][:], lhsT=wk_sb[:, wslice],
                                             rhs=xtile[:], start=st, stop=sp)
                    # RoPE: out = raw*cos + swap64(raw)*sin_signed, from PSUM
                    for h in range(HPC):
                        for nm, psrc, dst in (("q", qp[h], qt), ("k", kp[h], kt)):
                            t1 = rp.tile([128, SBK], f32, tag="t1",
                                         name=f"t1{nm}{sb}_{h}")
                            t2 = rp.tile([128, SBK], f32, tag="t2",
                                         name=f"t2{nm}{sb}_{h}")
                            nc.vector.tensor_mul(t1[:], psrc[:], cos_sb[:])
                            nc.vector.tensor_mul(t2[0:64, :], psrc[64:128, :],
                                                 sin_sb[0:64, :])
                            nc.vector.tensor_mul(t2[64:128, :], psrc[0:64, :],
                                                 sin_sb[64:128, :])
                            ds = slice(h * S + sb * SBK, h * S + (sb + 1) * SBK)
                            nc.vector.tensor_add(dst[:, ds], t1[:], t2[:])

            # masks loaded early; ot lives through C+D
            with tc.tile_pool(name="cpersist", bufs=1) as cpp:
                ot = cpp.tile([128, HPC * S], f32r, tag="ot")
                msk_sb = cpp.tile([128, 4 * SBK], f32r, tag="msk")
                nc.sync.dma_start(msk_sb[:], msk_d[:].bitcast(f32r))
                ones_col = msk_sb[:, 511:512]   # all-ones [128,1]
                ones_row = msk_sb[0:1, 0:128]   # all-ones [1,128]

                # ---------------- Phase B: V projection ----------------
                with tc.tile_pool(name="wv", bufs=1) as wvp, \
                     tc.tile_pool(name="xb", bufs=3) as xb:
                    wv_sb = wvp.tile([128, NEC * CW], f32r, tag="wv")
                    for sb in range(NSB):
                        vp = [ps.tile([128, CW], f32, tag="ps", name=f"vp{sb}_{_s}")
                              for _s in range(4)]
                        for ec in range(NEC):
                            xtile = xb.tile([128, SBK], f32r, tag="xb")
                            nc.sync.dma_start(
                                xtile[:],
                                xt_d[ec * 128:(ec + 1) * 128,
                                     sb * SBK:(sb + 1) * SBK].bitcast(f32r))
                            if sb == 0:
                                nc.sync.dma_start(
                                    wv_sb[:, ec * CW:(ec + 1) * CW],
                                    wv_d[ec * 128:(ec + 1) * 128, :].bitcast(f32r))
                            st, sp = (ec == 0), (ec == NEC - 1)
                            for sub in range(4):
                                nc.tensor.matmul(
                                    vp[sub][:],
                                    lhsT=xtile[:, sub * 128:(sub + 1) * 128],
                                    rhs=wv_sb[:, ec * CW:(ec + 1) * CW],
                                    start=st, stop=sp)
                        for sub in range(4):
                            stile = sb * 4 + sub
                            nc.scalar.copy(vt[:, stile * CW:(stile + 1) * CW],
                                           vp[sub][:])

                # ---------------- Phase C: attention per head ----------------
                with tc.tile_pool(name="pts", bufs=4) as ptp, \
                     tc.tile_pool(name="recs", bufs=2) as rcp, \
                     tc.tile_pool(name="bcs", bufs=2) as bcp:
                    for h in range(HPC):
                        for j in range(NSB):
                            nkt = 4 * j + 4  # causal: k tiles 0..4j+3
                            avp = ps.tile([128, SBK], f32, tag="ps",
                                          name=f"av{h}_{j}")
                            lfull = ps.tile([128, SBK], f32, tag="ps",
                                            name=f"l{h}_{j}")
                            lp = lfull[0:1, :]
                            qs = slice(h * S + j * SBK, h * S + (j + 1) * SBK)
                            for i in range(nkt):
                                sp_t = ps.tile([128, SBK], f32, tag="ps",
                                               name=f"s{h}_{j}_{i}")
                                ks = slice(h * S + i * 128, h * S + (i + 1) * 128)
                                nc.tensor.matmul(sp_t[:], lhsT=kt[:, ks],
                                                 rhs=qt[:, qs],
                                                 start=True, stop=True)
                                pt_sb = ptp.tile([128, SBK], f32r, tag="p",
                                                 name=f"p{h}_{j}_{i}")
                                nc.scalar.activation(pt_sb[:], sp_t[:], EXP,
                                                     scale=SCALE)
                                o_idx = i - 4 * j
                                if o_idx >= 0:  # diagonal-crossing tile
                                    nc.vector.tensor_mul(
                                        pt_sb[:], pt_sb[:],
                                        msk_sb[:, o_idx * SBK:(o_idx + 1) * SBK])
                                st, sp = (i == 0), (i == nkt - 1)
                                nc.tensor.matmul(
                                    avp[:],
                                    lhsT=vt[:, i * CW + h * HD:i * CW + (h + 1) * HD],
                                    rhs=pt_sb[:], start=st, stop=sp)
                                nc.tensor.matmul(lp, lhsT=ones_col,
                                                 rhs=pt_sb[:], start=st, stop=sp)
                            # 1/l = exp(-ln(l)); Ln/Exp share one ACT table set
                            lnt = rcp.tile([1, SBK], f32r, tag="rec",
                                           name=f"ln{h}_{j}")
                            nc.scalar.activation(lnt[:], lp, LN)
                            bcps = ps.tile([128, SBK], f32, tag="ps",
                                           name=f"bc{h}_{j}")
                            nc.tensor.matmul(bcps[:], lhsT=ones_row,
                                             rhs=lnt[:], start=True, stop=True)
                            bc_sb = bcp.tile([128, SBK], f32, tag="bcs",
                                             name=f"bcs{h}_{j}")
                            nc.scalar.activation(bc_sb[:], bcps[:], EXP,
                                                 scale=-1.0)
                            nc.vector.tensor_mul(ot[:, qs], avp[:], bc_sb[:])

                # ---------------- Phase D: output projection ----------------
                with tc.tile_pool(name="wo", bufs=1) as wop, \
                     tc.tile_pool(name="zsb", bufs=4) as zp:
                    wo_sb = wop.tile([128, HPC * EMBED], f32r, tag="wo")
                    for h in range(HPC):
                        nc.sync.dma_start(
                            wo_sb[:, h * EMBED:(h + 1) * EMBED],
                            wo_d[h * 128:(h + 1) * 128, :].bitcast(f32r))
                    for q_i in range(NST):
                        for eb in range(4):
                            zps = ps.tile([128, SBK], f32, tag="ps",
                                          name=f"z{q_i}_{eb}")
                            for h in range(HPC):
                                nc.tensor.matmul(
                                    zps[:],
                                    lhsT=ot[:, h * S + q_i * 128:
                                            h * S + (q_i + 1) * 128],
                                    rhs=wo_sb[:, h * EMBED + eb * SBK:
                                              h * EMBED + (eb + 1) * SBK],
                                    start=(h == 0), stop=(h == HPC - 1))
                            z_sb = zp.tile([128, SBK], f32, tag="zs",
                                           name=f"zs{q_i}_{eb}")
                            nc.vector.tensor_copy(z_sb[:], zps[:])
                            nc.sync.dma_start(
                                z_d[q_i * 128:(q_i + 1) * 128,
                                    eb * SBK:(eb + 1) * SBK], z_sb[:])

    nc.compile()
    return nc


def _host_tables():
    inv_freq = 1.0 / (ROPE_BASE ** (np.arange(0, HD, 2, dtype=np.float64) / HD))
    ang = np.arange(S, dtype=np.float64)[:, None] * inv_freq[None, :]  # [S, 64]
    cos = np.cos(ang)
    sin = np.sin(ang)
    cost = np.ascontiguousarray(
        np.concatenate([cos, cos], axis=1).T.astype(np.float32))  # [128, S]
    sints = np.ascontiguousarray(
        np.concatenate([-sin, sin], axis=1).T.astype(np.float32))
    kk = np.arange(128)[:, None]
    qq = np.arange(SBK)[None, :]
    masks = np.zeros((128, 4 * SBK), dtype=np.float32)
    for o in range(4):
        masks[:, o * SBK:(o + 1) * SBK] = (kk <= qq - o * 128).astype(np.float32)
    return cost, sints, masks


def _in_maps(x, Wq, Wk, Wv, Wo):
    cost, sints, masks = _host_tables()
    maps = []
    for c in range(N_CORES):
        b = c // 4
        h0 = (c % 4) * CW  # column offset of this core's 4 heads
        maps.append({
            "xt": np.ascontiguousarray(x[b].T),
            "wq": np.ascontiguousarray(Wq[:, h0:h0 + CW]),
            "wk": np.ascontiguousarray(Wk[:, h0:h0 + CW]),
            "wv": np.ascontiguousarray(Wv[:, h0:h0 + CW]),
            "wo": np.ascontiguousarray(Wo[h0:h0 + CW, :]),
            "cost": cost,
            "sints": sints,
            "masks": masks,
        })
    return maps


def kernel(x, Wq, Wk, Wv, Wo):
    from concourse.bass_utils import run_bass_kernel_spmd

    x = np.asarray(x, dtype=np.float32)
    Wq = np.asarray(Wq, dtype=np.float32)
    Wk = np.asarray(Wk, dtype=np.float32)
    Wv = np.asarray(Wv, dtype=np.float32)
    Wo = np.asarray(Wo, dtype=np.float32)

    if "nc" not in _CACHE:
        _CACHE["nc"] = _build_program()
    nc = _CACHE["nc"]

    res = run_bass_kernel_spmd(nc, _in_maps(x, Wq, Wk, Wv, Wo),
                               core_ids=list(range(N_CORES)))
    zs = [res.results[c]["z"] for c in range(N_CORES)]
    out = np.empty((B, S, EMBED), dtype=np.float32)
    out[0] = zs[0] + zs[1] + zs[2] + zs[3]
    out[1] = zs[4] + zs[5] + zs[6] + zs[7]
    return out


# revision 6
# speedup vs baseline: 1.1013x; 1.1013x over previous
"""MultiHeadAttention (RoPE, causal) Trainium2 kernel over 8 NeuronCores.

Sharding: batch (2) x head-groups (4 heads each) -> 8 cores.
Each core computes, for its batch b and 4 heads:
  Q^T,K^T = (Wq/Wk chunk)^T @ x^T   (RoPE applied on-chip)
  S^T tiles = K^T_tile contract-d Q^T, exp (no max-sub; scores ~N(0,1)),
  causal mask via precomputed 0/1 tiles,
  O^T = V contract-k P^T; row-sums l via ones-matmul into psum row 0;
  1/l via ACT Ln -> broadcast matmul -> ACT Exp(scale=-1)  (one table set);
  O^T normalized on the PSUM->SBUF path, then Z_partial = O @ Wo_chunk.
Host sums the 4 per-core partials of each batch.

All matmuls run in float32r (full-rate PE); storage fp32.
"""

import sys

if "/opt/trn_rl_repo" not in sys.path:
    sys.path.insert(0, "/opt/trn_rl_repo")

import numpy as np

EMBED = 2048
S = 2048
NH = 16
HD = 128
B = 2
N_CORES = 8
HPC = 4              # heads per core
CW = HPC * HD        # 512: per-core projection width
SBK = 512            # s block width
NSB = S // SBK       # 4
NEC = EMBED // 128   # 16 e-chunks
NST = S // 128       # 16 s tiles / q tiles / k tiles
ROPE_BASE = 10000.0
SCALE = 1.0 / float(np.sqrt(HD))

_CACHE = {}


def _build_program():
    import concourse.bacc as bacc
    import concourse.mybir as mybir
    import concourse.tile as tile

    f32 = mybir.dt.float32
    f32r = mybir.dt.float32r
    EXP = mybir.ActivationFunctionType.Exp
    LN = mybir.ActivationFunctionType.Ln

    nc = bacc.Bacc("TRN2", target_bir_lowering=False, debug=False,
                   num_devices=N_CORES)

    xt_d = nc.dram_tensor("xt", [EMBED, S], f32, kind="ExternalInput").ap()
    wq_d = nc.dram_tensor("wq", [EMBED, CW], f32, kind="ExternalInput").ap()
    wk_d = nc.dram_tensor("wk", [EMBED, CW], f32, kind="ExternalInput").ap()
    wv_d = nc.dram_tensor("wv", [EMBED, CW], f32, kind="ExternalInput").ap()
    wo_d = nc.dram_tensor("wo", [CW, EMBED], f32, kind="ExternalInput").ap()
    cos_d = nc.dram_tensor("cost", [HD, S], f32, kind="ExternalInput").ap()
    sin_d = nc.dram_tensor("sints", [HD, S], f32, kind="ExternalInput").ap()
    msk_d = nc.dram_tensor("masks", [128, 4 * SBK], f32, kind="ExternalInput").ap()
    z_d = nc.dram_tensor("z", [S, EMBED], f32, kind="ExternalOutput").ap()

    with tile.TileContext(nc) as tc, \
         nc.allow_low_precision(reason="fp32r attention pipeline"):
        with tc.tile_pool(name="persist", bufs=1) as pp, \
             tc.tile_pool(name="ps", bufs=8, space="PSUM") as ps:
            qt = pp.tile([128, HPC * S], f32r, tag="qt")   # Q^T rope, per head
            kt = pp.tile([128, HPC * S], f32r, tag="kt")   # K^T rope, per head
            vt = pp.tile([128, NST * CW], f32r, tag="vt")  # V, [s-tile, 4 heads]

            # ---------------- Phase A: Q/K projections + RoPE ----------------
            with tc.tile_pool(name="wqk", bufs=1) as wp, \
                 tc.tile_pool(name="cossin", bufs=2) as cs, \
                 tc.tile_pool(name="xa", bufs=2) as xa, \
                 tc.tile_pool(name="ropetmp", bufs=1) as rp:
                wq_sb = wp.tile([128, NEC * CW], f32r, tag="wq")
                wk_sb = wp.tile([128, NEC * CW], f32r, tag="wk")

                for sb in range(NSB):
                    cos_sb = cs.tile([128, SBK], f32, tag="cos")
                    sin_sb = cs.tile([128, SBK], f32, tag="sin")
                    ss = slice(sb * SBK, (sb + 1) * SBK)
                    nc.sync.dma_start(cos_sb[:], cos_d[:, ss])
                    nc.sync.dma_start(sin_sb[:], sin_d[:, ss])
                    qp = [ps.tile([128, SBK], f32, tag="ps", name=f"qp{sb}_{_h}")
                          for _h in range(HPC)]
                    kp = [ps.tile([128, SBK], f32, tag="ps", name=f"kp{sb}_{_h}")
                          for _h in range(HPC)]
                    for g in range(NEC // 4):  # groups of 4 e-chunks
                        # one DMA brings 4 e-chunks of x^T for this s block
                        xtile = xa.tile([128, 4 * SBK], f32r, tag="x",
                                        name=f"x{sb}_{g}")
                        src_ap = xt_d[g * 512:(g + 1) * 512,
                                      sb * SBK:(sb + 1) * SBK]
                        nc.sync.dma_start(
                            xtile[:].rearrange("p (c s) -> p c s", s=SBK),
                            src_ap.rearrange("(c p) s -> p c s",
                                             p=128).bitcast(f32r))
                        if sb == 0:
                            nc.sync.dma_start(
                                wq_sb[:, g * 4 * CW:(g + 1) * 4 * CW].rearrange(
                                    "p (c m) -> p c m", m=CW),
                                wq_d[g * 512:(g + 1) * 512, :].rearrange(
                                    "(c p) m -> p c m", p=128).bitcast(f32r))
                            nc.sync.dma_start(
                                wk_sb[:, g * 4 * CW:(g + 1) * 4 * CW].rearrange(
                                    "p (c m) -> p c m", m=CW),
                                wk_d[g * 512:(g + 1) * 512, :].rearrange(
                                    "(c p) m -> p c m", p=128).bitcast(f32r))
                        for el in range(4):
                            ec = g * 4 + el
                            st, sp = (ec == 0), (ec == NEC - 1)
                            xv = xtile[:, el * SBK:(el + 1) * SBK]
                            for h in range(HPC):
                                wslice = slice(ec * CW + h * HD,
                                               ec * CW + (h + 1) * HD)
                                nc.tensor.matmul(qp[h][:], lhsT=wq_sb[:, wslice],
                                                 rhs=xv, start=st, stop=sp)
                                nc.tensor.matmul(kp[h][:], lhsT=wk_sb[:, wslice],
                                                 rhs=xv, start=st, stop=sp)
                    # RoPE: out = raw*cos + swap64(raw)*sin_signed, from PSUM
                    for h in range(HPC):
                        for nm, psrc, dst in (("q", qp[h], qt), ("k", kp[h], kt)):
                            t1 = rp.tile([128, SBK], f32, tag="t1",
                                         name=f"t1{nm}{sb}_{h}")
                            t2 = rp.tile([128, SBK], f32, tag="t2",
                                         name=f"t2{nm}{sb}_{h}")
                            nc.vector.tensor_mul(t1[:], psrc[:], cos_sb[:])
                            nc.vector.tensor_mul(t2[0:64, :], psrc[64:128, :],
                                                 sin_sb[0:64, :])
                            nc.vector.tensor_mul(t2[64:128, :], psrc[0:64, :],
                                                 sin_sb[64:128, :])
                            ds = slice(h * S + sb * SBK, h * S + (sb + 1) * SBK)
                            nc.vector.tensor_add(dst[:, ds], t1[:], t2[:])

            # masks loaded early; ot lives through C+D
            with tc.tile_pool(name="cpersist", bufs=1) as cpp:
                ot = cpp.tile([128, HPC * S], f32r, tag="ot")
                msk_sb = cpp.tile([128, 4 * SBK], f32r, tag="msk")
                nc.sync.dma_start(msk_sb[:], msk_d[:].bitcast(f32r))
                ones_col = msk_sb[:, 511:512]   # all-ones [128,1]
                ones_row = msk_sb[0:1, 0:128]   # all-ones [1,128]

                # ---------------- Phase B: V projection ----------------
                with tc.tile_pool(name="wv", bufs=1) as wvp, \
                     tc.tile_pool(name="xb", bufs=2) as xb:
                    wv_sb = wvp.tile([128, NEC * CW], f32r, tag="wv")
                    for sb in range(NSB):
                        vp = [ps.tile([128, CW], f32, tag="ps", name=f"vp{sb}_{_s}")
                              for _s in range(4)]
                        for g in range(NEC // 4):
                            xtile = xb.tile([128, 4 * SBK], f32r, tag="xb",
                                            name=f"xb{sb}_{g}")
                            src_ap = xt_d[g * 512:(g + 1) * 512,
                                          sb * SBK:(sb + 1) * SBK]
                            nc.sync.dma_start(
                                xtile[:].rearrange("p (c s) -> p c s", s=SBK),
                                src_ap.rearrange("(c p) s -> p c s",
                                                 p=128).bitcast(f32r))
                            if sb == 0:
                                nc.sync.dma_start(
                                    wv_sb[:, g * 4 * CW:(g + 1) * 4 * CW].rearrange(
                                        "p (c m) -> p c m", m=CW),
                                    wv_d[g * 512:(g + 1) * 512, :].rearrange(
                                        "(c p) m -> p c m", p=128).bitcast(f32r))
                            for el in range(4):
                                ec = g * 4 + el
                                st, sp = (ec == 0), (ec == NEC - 1)
                                for sub in range(4):
                                    nc.tensor.matmul(
                                        vp[sub][:],
                                        lhsT=xtile[:, el * SBK + sub * 128:
                                                   el * SBK + (sub + 1) * 128],
                                        rhs=wv_sb[:, ec * CW:(ec + 1) * CW],
                                        start=st, stop=sp)
                        for sub in range(4):
                            stile = sb * 4 + sub
                            nc.scalar.copy(vt[:, stile * CW:(stile + 1) * CW],
                                           vp[sub][:])

                # ---------------- Phase C: attention per head ----------------
                with tc.tile_pool(name="pts", bufs=4) as ptp, \
                     tc.tile_pool(name="recs", bufs=2) as rcp, \
                     tc.tile_pool(name="bcs", bufs=2) as bcp:
                    for h in range(HPC):
                        for j in range(NSB):
                            nkt = 4 * j + 4  # causal: k tiles 0..4j+3
                            avp = ps.tile([128, SBK], f32, tag="ps",
                                          name=f"av{h}_{j}")
                            lfull = ps.tile([128, SBK], f32, tag="ps",
                                            name=f"l{h}_{j}")
                            lp = lfull[0:1, :]
                            qs = slice(h * S + j * SBK, h * S + (j + 1) * SBK)
                            for i in range(nkt):
                                sp_t = ps.tile([128, SBK], f32, tag="ps",
                                               name=f"s{h}_{j}_{i}")
                                ks = slice(h * S + i * 128, h * S + (i + 1) * 128)
                                nc.tensor.matmul(sp_t[:], lhsT=kt[:, ks],
                                                 rhs=qt[:, qs],
                                                 start=True, stop=True)
                                pt_sb = ptp.tile([128, SBK], f32r, tag="p",
                                                 name=f"p{h}_{j}_{i}")
                                nc.scalar.activation(pt_sb[:], sp_t[:], EXP,
                                                     scale=SCALE)
                                o_idx = i - 4 * j
                                if o_idx >= 0:  # diagonal-crossing tile
                                    nc.vector.tensor_mul(
                                        pt_sb[:], pt_sb[:],
                                        msk_sb[:, o_idx * SBK:(o_idx + 1) * SBK])
                                st, sp = (i == 0), (i == nkt - 1)
                                nc.tensor.matmul(
                                    avp[:],
                                    lhsT=vt[:, i * CW + h * HD:i * CW + (h + 1) * HD],
                                    rhs=pt_sb[:], start=st, stop=sp)
                                nc.tensor.matmul(lp, lhsT=ones_col,
                                                 rhs=pt_sb[:], start=st, stop=sp)
                            # 1/l = exp(-ln(l)); Ln/Exp share one ACT table set
                            lnt = rcp.tile([1, SBK], f32r, tag="rec",
                                           name=f"ln{h}_{j}")
                            nc.scalar.activation(lnt[:], lp, LN)
                            bcps = ps.tile([128, SBK], f32, tag="ps",
                                           name=f"bc{h}_{j}")
                            nc.tensor.matmul(bcps[:], lhsT=ones_row,
                                             rhs=lnt[:], start=True, stop=True)
                            bc_sb = bcp.tile([128, SBK], f32, tag="bcs",
                                             name=f"bcs{h}_{j}")
                            nc.scalar.activation(bc_sb[:], bcps[:], EXP,
                                                 scale=-1.0)
                            nc.vector.tensor_mul(ot[:, qs], avp[:], bc_sb[:])

                # ---------------- Phase D: output projection ----------------
                with tc.tile_pool(name="wo", bufs=1) as wop, \
                     tc.tile_pool(name="zsb", bufs=2) as zp:
                    wo_sb = wop.tile([128, HPC * EMBED], f32r, tag="wo")
                    for h in range(HPC):
                        nc.sync.dma_start(
                            wo_sb[:, h * EMBED:(h + 1) * EMBED],
                            wo_d[h * 128:(h + 1) * 128, :].bitcast(f32r))
                    for q_i in range(NST):
                        z_sb = zp.tile([128, EMBED], f32, tag="zs",
                                       name=f"zs{q_i}")
                        for eb in range(4):
                            zps = ps.tile([128, SBK], f32, tag="ps",
                                          name=f"z{q_i}_{eb}")
                            for h in range(HPC):
                                nc.tensor.matmul(
                                    zps[:],
                                    lhsT=ot[:, h * S + q_i * 128:
                                            h * S + (q_i + 1) * 128],
                                    rhs=wo_sb[:, h * EMBED + eb * SBK:
                                              h * EMBED + (eb + 1) * SBK],
                                    start=(h == 0), stop=(h == HPC - 1))
                            nc.vector.tensor_copy(
                                z_sb[:, eb * SBK:(eb + 1) * SBK], zps[:])
                        nc.sync.dma_start(z_d[q_i * 128:(q_i + 1) * 128, :],
                                          z_sb[:])

    nc.compile()
    return nc


def _host_tables():
    inv_freq = 1.0 / (ROPE_BASE ** (np.arange(0, HD, 2, dtype=np.float64) / HD))
    ang = np.arange(S, dtype=np.float64)[:, None] * inv_freq[None, :]  # [S, 64]
    cos = np.cos(ang)
    sin = np.sin(ang)
    cost = np.ascontiguousarray(
        np.concatenate([cos, cos], axis=1).T.astype(np.float32))  # [128, S]
    sints = np.ascontiguousarray(
        np.concatenate([-sin, sin], axis=1).T.astype(np.float32))
    kk = np.arange(128)[:, None]
    qq = np.arange(SBK)[None, :]
    masks = np.zeros((128, 4 * SBK), dtype=np.float32)
    for o in range(4):
        masks[:, o * SBK:(o + 1) * SBK] = (kk <= qq - o * 128).astype(np.float32)
    return cost, sints, masks


def _in_maps(x, Wq, Wk, Wv, Wo):
    cost, sints, masks = _host_tables()
    maps = []
    for c in range(N_CORES):
        b = c // 4
        h0 = (c % 4) * CW  # column offset of this core's 4 heads
        maps.append({
            "xt": np.ascontiguousarray(x[b].T),
            "wq": np.ascontiguousarray(Wq[:, h0:h0 + CW]),
            "wk": np.ascontiguousarray(Wk[:, h0:h0 + CW]),
            "wv": np.ascontiguousarray(Wv[:, h0:h0 + CW]),
            "wo": np.ascontiguousarray(Wo[h0:h0 + CW, :]),
            "cost": cost,
            "sints": sints,
            "masks": masks,
        })
    return maps


def kernel(x, Wq, Wk, Wv, Wo):
    from concourse.bass_utils import run_bass_kernel_spmd

    x = np.asarray(x, dtype=np.float32)
    Wq = np.asarray(Wq, dtype=np.float32)
    Wk = np.asarray(Wk, dtype=np.float32)
    Wv = np.asarray(Wv, dtype=np.float32)
    Wo = np.asarray(Wo, dtype=np.float32)

    if "nc" not in _CACHE:
        _CACHE["nc"] = _build_program()
    nc = _CACHE["nc"]

    res = run_bass_kernel_spmd(nc, _in_maps(x, Wq, Wk, Wv, Wo),
                               core_ids=list(range(N_CORES)))
    zs = [res.results[c]["z"] for c in range(N_CORES)]
    out = np.empty((B, S, EMBED), dtype=np.float32)
    out[0] = zs[0] + zs[1] + zs[2] + zs[3]
    out[1] = zs[4] + zs[5] + zs[6] + zs[7]
    return out


# revision 7
# speedup vs baseline: 1.3565x; 1.2318x over previous
"""MultiHeadAttention (RoPE, causal) Trainium2 kernel over 8 NeuronCores.

Sharding: batch (2) x head-groups (4 heads each) -> 8 cores.
Each core computes, for its batch b and 4 heads:
  Q^T,K^T = (Wq/Wk chunk)^T @ x^T   (RoPE applied on-chip)
  S^T tiles = K^T_tile contract-d Q^T, exp (no max-sub; scores ~N(0,1)),
  causal mask via precomputed 0/1 tiles,
  O^T = V contract-k P^T; row-sums l via ones-matmul into psum row 0.
  Normalization is deferred and batched: 1/l = exp(-ln l) on ACT (one
  table-set switch), broadcast over partitions with a K=1 matmul, then
  one fused multiply into O^T. Finally Z_partial = O @ Wo_chunk.
Host sums the 4 per-core partials of each batch.

All matmuls run in float32r (full-rate PE); storage fp32.
"""

import sys

if "/opt/trn_rl_repo" not in sys.path:
    sys.path.insert(0, "/opt/trn_rl_repo")

import numpy as np

EMBED = 2048
S = 2048
NH = 16
HD = 128
B = 2
N_CORES = 8
HPC = 4              # heads per core
CW = HPC * HD        # 512: per-core projection width
SBK = 512            # s block width
NSB = S // SBK       # 4
NEC = EMBED // 128   # 16 e-chunks
NST = S // 128       # 16 s tiles / q tiles / k tiles
ROPE_BASE = 10000.0
SCALE = 1.0 / float(np.sqrt(HD))

_CACHE = {}


def _build_program():
    import concourse.bacc as bacc
    import concourse.mybir as mybir
    import concourse.tile as tile

    f32 = mybir.dt.float32
    f32r = mybir.dt.float32r
    EXP = mybir.ActivationFunctionType.Exp
    LN = mybir.ActivationFunctionType.Ln

    nc = bacc.Bacc("TRN2", target_bir_lowering=False, debug=False,
                   num_devices=N_CORES)

    xt_d = nc.dram_tensor("xt", [EMBED, S], f32, kind="ExternalInput").ap()
    wq_d = nc.dram_tensor("wq", [EMBED, CW], f32, kind="ExternalInput").ap()
    wk_d = nc.dram_tensor("wk", [EMBED, CW], f32, kind="ExternalInput").ap()
    wv_d = nc.dram_tensor("wv", [EMBED, CW], f32, kind="ExternalInput").ap()
    wo_d = nc.dram_tensor("wo", [CW, EMBED], f32, kind="ExternalInput").ap()
    cos_d = nc.dram_tensor("cost", [HD, S], f32, kind="ExternalInput").ap()
    sin_d = nc.dram_tensor("sints", [HD, S], f32, kind="ExternalInput").ap()
    msk_d = nc.dram_tensor("masks", [128, 4 * SBK], f32, kind="ExternalInput").ap()
    z_d = nc.dram_tensor("z", [S, EMBED], f32, kind="ExternalOutput").ap()

    XG = 8           # x tiles per s-block (2 e-chunks each)
    XW = 2 * SBK     # x tile width

    with tile.TileContext(nc) as tc, \
         nc.allow_low_precision(reason="fp32r attention pipeline"):
        with tc.tile_pool(name="persist", bufs=1) as pp, \
             tc.tile_pool(name="ps", bufs=8, space="PSUM") as ps:
            qt = pp.tile([128, HPC * S], f32r, tag="qt")   # Q^T rope, per head
            kt = pp.tile([128, HPC * S], f32r, tag="kt")   # K^T rope, per head

            # ---------------- Phase A: Q/K projections + RoPE ----------------
            # Per s-block: load 8 x-tiles (held in SBUF), sweep Q over all
            # e-chunks, then sweep K reusing the same x-tiles.  Q's RoPE
            # (DVE) hides under the K sweep and vice versa.
            with tc.tile_pool(name="wqk", bufs=1) as wp, \
                 tc.tile_pool(name="cossin", bufs=2) as cs, \
                 tc.tile_pool(name="xa", bufs=XG) as xa, \
                 tc.tile_pool(name="ropetmp", bufs=1) as rp:
                wq_sb = wp.tile([128, NEC * CW], f32r, tag="wq")
                wk_sb = wp.tile([128, NEC * CW], f32r, tag="wk")

                def rope(psrc, dst, cos_sb, sin_sb, sb, h, nm):
                    t1 = rp.tile([128, SBK], f32, tag="t1", name=f"t1{nm}{sb}_{h}")
                    t2 = rp.tile([128, SBK], f32, tag="t2", name=f"t2{nm}{sb}_{h}")
                    nc.vector.tensor_mul(t1[:], psrc[:], cos_sb[:])
                    nc.vector.tensor_mul(t2[0:64, :], psrc[64:128, :],
                                         sin_sb[0:64, :])
                    nc.vector.tensor_mul(t2[64:128, :], psrc[0:64, :],
                                         sin_sb[64:128, :])
                    ds = slice(h * S + sb * SBK, h * S + (sb + 1) * SBK)
                    nc.vector.tensor_add(dst[:, ds], t1[:], t2[:])

                for sb in range(NSB):
                    cos_sb = cs.tile([128, SBK], f32, tag="cos", name=f"cos{sb}")
                    sin_sb = cs.tile([128, SBK], f32, tag="sin", name=f"sin{sb}")
                    ss = slice(sb * SBK, (sb + 1) * SBK)
                    nc.sync.dma_start(cos_sb[:], cos_d[:, ss])
                    nc.sync.dma_start(sin_sb[:], sin_d[:, ss])
                    xts = []
                    for g in range(XG):
                        xt_g = xa.tile([128, XW], f32r, tag="x",
                                       name=f"x{sb}_{g}")
                        src_ap = xt_d[g * 256:(g + 1) * 256, ss]
                        nc.sync.dma_start(
                            xt_g[:].rearrange("p (c s) -> p c s", s=SBK),
                            src_ap.rearrange("(c p) s -> p c s",
                                             p=128).bitcast(f32r))
                        xts.append(xt_g)
                    # -------- Q sweep --------
                    qp = [ps.tile([128, SBK], f32, tag="ps", name=f"qp{sb}_{_h}")
                          for _h in range(HPC)]
                    for g in range(XG):
                        if sb == 0:
                            nc.sync.dma_start(
                                wq_sb[:, g * 2 * CW:(g + 1) * 2 * CW].rearrange(
                                    "p (c m) -> p c m", m=CW),
                                wq_d[g * 256:(g + 1) * 256, :].rearrange(
                                    "(c p) m -> p c m", p=128).bitcast(f32r))
                        for el in range(2):
                            ec = 2 * g + el
                            st, sp = (ec == 0), (ec == NEC - 1)
                            xv = xts[g][:, el * SBK:(el + 1) * SBK]
                            for h in range(HPC):
                                wsl = slice(ec * CW + h * HD,
                                            ec * CW + (h + 1) * HD)
                                nc.tensor.matmul(qp[h][:], lhsT=wq_sb[:, wsl],
                                                 rhs=xv, start=st, stop=sp)
                    for h in range(HPC):
                        rope(qp[h], qt, cos_sb, sin_sb, sb, h, "q")
                    # -------- K sweep --------
                    kp = [ps.tile([128, SBK], f32, tag="ps", name=f"kp{sb}_{_h}")
                          for _h in range(HPC)]
                    for g in range(XG):
                        if sb == 0:
                            nc.sync.dma_start(
                                wk_sb[:, g * 2 * CW:(g + 1) * 2 * CW].rearrange(
                                    "p (c m) -> p c m", m=CW),
                                wk_d[g * 256:(g + 1) * 256, :].rearrange(
                                    "(c p) m -> p c m", p=128).bitcast(f32r))
                        for el in range(2):
                            ec = 2 * g + el
                            st, sp = (ec == 0), (ec == NEC - 1)
                            xv = xts[g][:, el * SBK:(el + 1) * SBK]
                            for h in range(HPC):
                                wsl = slice(ec * CW + h * HD,
                                            ec * CW + (h + 1) * HD)
                                nc.tensor.matmul(kp[h][:], lhsT=wk_sb[:, wsl],
                                                 rhs=xv, start=st, stop=sp)
                    for h in range(HPC):
                        rope(kp[h], kt, cos_sb, sin_sb, sb, h, "k")

            # vt/ot/masks live from phase B through D
            with tc.tile_pool(name="cpersist", bufs=1) as cpp:
                vt = cpp.tile([128, NST * CW], f32r, tag="vt")
                ot = cpp.tile([128, HPC * S], f32r, tag="ot")
                msk_sb = cpp.tile([128, 4 * SBK], f32r, tag="msk")
                nc.sync.dma_start(msk_sb[:], msk_d[:].bitcast(f32r))
                ones_col = msk_sb[:, 511:512]   # all-ones [128,1]
                ones_row = msk_sb[0:1, 0:128]   # all-ones [1,128]

                # ---------------- Phase B: V projection ----------------
                with tc.tile_pool(name="wv", bufs=1) as wvp, \
                     tc.tile_pool(name="xb", bufs=2) as xb:
                    wv_sb = wvp.tile([128, NEC * CW], f32r, tag="wv")
                    for sb in range(NSB):
                        vp = [ps.tile([128, CW], f32, tag="ps",
                                      name=f"vp{sb}_{_s}") for _s in range(4)]
                        for g in range(NEC // 4):
                            xtile = xb.tile([128, 4 * SBK], f32r, tag="xb",
                                            name=f"xb{sb}_{g}")
                            src_ap = xt_d[g * 512:(g + 1) * 512,
                                          sb * SBK:(sb + 1) * SBK]
                            nc.sync.dma_start(
                                xtile[:].rearrange("p (c s) -> p c s", s=SBK),
                                src_ap.rearrange("(c p) s -> p c s",
                                                 p=128).bitcast(f32r))
                            if sb == 0:
                                nc.sync.dma_start(
                                    wv_sb[:, g * 4 * CW:(g + 1) * 4 * CW].rearrange(
                                        "p (c m) -> p c m", m=CW),
                                    wv_d[g * 512:(g + 1) * 512, :].rearrange(
                                        "(c p) m -> p c m", p=128).bitcast(f32r))
                            for el in range(4):
                                ec = g * 4 + el
                                st, sp = (ec == 0), (ec == NEC - 1)
                                for sub in range(4):
                                    nc.tensor.matmul(
                                        vp[sub][:],
                                        lhsT=xtile[:, el * SBK + sub * 128:
                                                   el * SBK + (sub + 1) * 128],
                                        rhs=wv_sb[:, ec * CW:(ec + 1) * CW],
                                        start=st, stop=sp)
                        for sub in range(4):
                            stile = sb * 4 + sub
                            nc.scalar.copy(vt[:, stile * CW:(stile + 1) * CW],
                                           vp[sub][:])

                # ---------------- Phase C: attention per head ----------------
                with tc.tile_pool(name="lall", bufs=1) as lap:
                    l_all = lap.tile([1, 16 * SBK], f32, tag="lall")
                    with tc.tile_pool(name="pts", bufs=4) as ptp:
                        for h in range(HPC):
                            for j in range(NSB):
                                blk = h * NSB + j
                                nkt = 4 * j + 4  # causal: k tiles 0..4j+3
                                avp = ps.tile([128, SBK], f32, tag="ps",
                                              name=f"av{h}_{j}")
                                lfull = ps.tile([128, SBK], f32, tag="ps",
                                                name=f"l{h}_{j}")
                                lp = lfull[0:1, :]
                                qs = slice(h * S + j * SBK, h * S + (j + 1) * SBK)
                                for i in range(nkt):
                                    sp_t = ps.tile([128, SBK], f32, tag="ps",
                                                   name=f"s{h}_{j}_{i}")
                                    ks = slice(h * S + i * 128,
                                               h * S + (i + 1) * 128)
                                    nc.tensor.matmul(sp_t[:], lhsT=kt[:, ks],
                                                     rhs=qt[:, qs],
                                                     start=True, stop=True)
                                    pt_sb = ptp.tile([128, SBK], f32r, tag="p",
                                                     name=f"p{h}_{j}_{i}")
                                    nc.scalar.activation(pt_sb[:], sp_t[:], EXP,
                                                         scale=SCALE)
                                    o_idx = i - 4 * j
                                    if o_idx >= 0:  # diagonal-crossing tile
                                        nc.vector.tensor_mul(
                                            pt_sb[:], pt_sb[:],
                                            msk_sb[:, o_idx * SBK:
                                                   (o_idx + 1) * SBK])
                                    st, sp = (i == 0), (i == nkt - 1)
                                    nc.tensor.matmul(
                                        avp[:],
                                        lhsT=vt[:, i * CW + h * HD:
                                                i * CW + (h + 1) * HD],
                                        rhs=pt_sb[:], start=st, stop=sp)
                                    nc.tensor.matmul(lp, lhsT=ones_col,
                                                     rhs=pt_sb[:],
                                                     start=st, stop=sp)
                                # stash unnormalized O^T and the row-sums
                                nc.vector.tensor_copy(ot[:, qs], avp[:])
                                nc.vector.tensor_copy(
                                    l_all[0:1, blk * SBK:(blk + 1) * SBK], lp)

                    # batched normalization: 1/l = exp(-ln l)
                    with tc.tile_pool(name="lnt", bufs=2) as lnp, \
                         tc.tile_pool(name="bcs", bufs=2) as bcp:
                        for h in range(HPC):
                            lnt = lnp.tile([1, NSB * SBK], f32r, tag="lnt",
                                           name=f"lnt{h}")
                            hs = slice(h * NSB * SBK, (h + 1) * NSB * SBK)
                            nc.scalar.activation(lnt[:], l_all[0:1, hs], LN)
                            for j in range(NSB):
                                bcps = ps.tile([128, SBK], f32, tag="ps",
                                               name=f"bc{h}_{j}")
                                nc.tensor.matmul(
                                    bcps[:], lhsT=ones_row,
                                    rhs=lnt[0:1, j * SBK:(j + 1) * SBK],
                                    start=True, stop=True)
                                bc_sb = bcp.tile([128, SBK], f32, tag="bcs",
                                                 name=f"bcs{h}_{j}")
                                nc.scalar.activation(bc_sb[:], bcps[:], EXP,
                                                     scale=-1.0)
                                qs = slice(h * S + j * SBK,
                                           h * S + (j + 1) * SBK)
                                nc.vector.tensor_mul(ot[:, qs], ot[:, qs],
                                                     bc_sb[:])

                # ---------------- Phase D: output projection ----------------
                with tc.tile_pool(name="wo", bufs=1) as wop, \
                     tc.tile_pool(name="zsb", bufs=2) as zp:
                    wo_sb = wop.tile([128, HPC * EMBED], f32r, tag="wo")
                    for h in range(HPC):
                        nc.sync.dma_start(
                            wo_sb[:, h * EMBED:(h + 1) * EMBED],
                            wo_d[h * 128:(h + 1) * 128, :].bitcast(f32r))
                    for q_i in range(NST):
                        z_sb = zp.tile([128, EMBED], f32, tag="zs",
                                       name=f"zs{q_i}")
                        for eb in range(4):
                            zps = ps.tile([128, SBK], f32, tag="ps",
                                          name=f"z{q_i}_{eb}")
                            for h in range(HPC):
                                nc.tensor.matmul(
                                    zps[:],
                                    lhsT=ot[:, h * S + q_i * 128:
                                            h * S + (q_i + 1) * 128],
                                    rhs=wo_sb[:, h * EMBED + eb * SBK:
                                              h * EMBED + (eb + 1) * SBK],
                                    start=(h == 0), stop=(h == HPC - 1))
                            nc.vector.tensor_copy(
                                z_sb[:, eb * SBK:(eb + 1) * SBK], zps[:])
                        nc.sync.dma_start(z_d[q_i * 128:(q_i + 1) * 128, :],
                                          z_sb[:])

    nc.compile()
    return nc


def _host_tables():
    inv_freq = 1.0 / (ROPE_BASE ** (np.arange(0, HD, 2, dtype=np.float64) / HD))
    ang = np.arange(S, dtype=np.float64)[:, None] * inv_freq[None, :]  # [S, 64]
    cos = np.cos(ang)
    sin = np.sin(ang)
    cost = np.ascontiguousarray(
        np.concatenate([cos, cos], axis=1).T.astype(np.float32))  # [128, S]
    sints = np.ascontiguousarray(
        np.concatenate([-sin, sin], axis=1).T.astype(np.float32))
    kk = np.arange(128)[:, None]
    qq = np.arange(SBK)[None, :]
    masks = np.zeros((128, 4 * SBK), dtype=np.float32)
    for o in range(4):
        masks[:, o * SBK:(o + 1) * SBK] = (kk <= qq - o * 128).astype(np.float32)
    return cost, sints, masks


def _in_maps(x, Wq, Wk, Wv, Wo):
    cost, sints, masks = _host_tables()
    maps = []
    for c in range(N_CORES):
        b = c // 4
        h0 = (c % 4) * CW  # column offset of this core's 4 heads
        maps.append({
            "xt": np.ascontiguousarray(x[b].T),
            "wq": np.ascontiguousarray(Wq[:, h0:h0 + CW]),
            "wk": np.ascontiguousarray(Wk[:, h0:h0 + CW]),
            "wv": np.ascontiguousarray(Wv[:, h0:h0 + CW]),
            "wo": np.ascontiguousarray(Wo[h0:h0 + CW, :]),
            "cost": cost,
            "sints": sints,
            "masks": masks,
        })
    return maps


def kernel(x, Wq, Wk, Wv, Wo):
    from concourse.bass_utils import run_bass_kernel_spmd

    x = np.asarray(x, dtype=np.float32)
    Wq = np.asarray(Wq, dtype=np.float32)
    Wk = np.asarray(Wk, dtype=np.float32)
    Wv = np.asarray(Wv, dtype=np.float32)
    Wo = np.asarray(Wo, dtype=np.float32)

    if "nc" not in _CACHE:
        _CACHE["nc"] = _build_program()
    nc = _CACHE["nc"]

    res = run_bass_kernel_spmd(nc, _in_maps(x, Wq, Wk, Wv, Wo),
                               core_ids=list(range(N_CORES)))
    zs = [res.results[c]["z"] for c in range(N_CORES)]
    out = np.empty((B, S, EMBED), dtype=np.float32)
    out[0] = zs[0] + zs[1] + zs[2] + zs[3]
    out[1] = zs[4] + zs[5] + zs[6] + zs[7]
    return out


# revision 11
# speedup vs baseline: 1.4907x; 1.0989x over previous
"""MultiHeadAttention (RoPE, causal) Trainium2 kernel over 8 NeuronCores.

Sharding: batch (2) x head-groups (4 heads each) -> 8 cores.
Each core computes, for its batch b and 4 heads:
  Q^T,K^T = (Wq/Wk chunk)^T @ x^T   (RoPE applied on-chip)
  S^T tiles = K^T_tile contract-d Q^T, exp (no max-sub; scores ~N(0,1)),
  causal mask via precomputed 0/1 tiles,
  O^T = V contract-k P^T; row-sums l via ones-matmul into psum row 0.
  Normalization is deferred and batched: 1/l = exp(-ln l) on ACT (one
  table-set switch), broadcast over partitions with a K=1 matmul, then
  one fused multiply into O^T. Finally Z_partial = O @ Wo_chunk.
Host sums the 4 per-core partials of each batch.

All matmuls run in float32r (full-rate PE); storage fp32.
Phases share one 8-bank PSUM pool and one x-tile pool (A's projection
sweeps and B's V sweep) so DMA prefetch flows across phase boundaries.
"""

import sys

if "/opt/trn_rl_repo" not in sys.path:
    sys.path.insert(0, "/opt/trn_rl_repo")

import numpy as np

EMBED = 2048
S = 2048
NH = 16
HD = 128
B = 2
N_CORES = 8
HPC = 4              # heads per core
CW = HPC * HD        # 512: per-core projection width
SBK = 512            # s block width
NSB = S // SBK       # 4
NEC = EMBED // 128   # 16 e-chunks
NST = S // 128       # 16 s tiles / q tiles / k tiles
ROPE_BASE = 10000.0
SCALE = 1.0 / float(np.sqrt(HD))

_CACHE = {}


def _build_program():
    import concourse.bacc as bacc
    import concourse.mybir as mybir
    import concourse.tile as tile

    f32 = mybir.dt.float32
    f32r = mybir.dt.float32r
    EXP = mybir.ActivationFunctionType.Exp
    LN = mybir.ActivationFunctionType.Ln

    nc = bacc.Bacc("TRN2", target_bir_lowering=False, debug=False,
                   num_devices=N_CORES)

    xt_d = nc.dram_tensor("xt", [EMBED, S], f32, kind="ExternalInput").ap()
    wq_d = nc.dram_tensor("wq", [EMBED, CW], f32, kind="ExternalInput").ap()
    wk_d = nc.dram_tensor("wk", [EMBED, CW], f32, kind="ExternalInput").ap()
    wv_d = nc.dram_tensor("wv", [EMBED, CW], f32, kind="ExternalInput").ap()
    wo_d = nc.dram_tensor("wo", [CW, EMBED], f32, kind="ExternalInput").ap()
    cos_d = nc.dram_tensor("cost", [HD, S], f32, kind="ExternalInput").ap()
    sin_d = nc.dram_tensor("sints", [HD, S], f32, kind="ExternalInput").ap()
    msk_d = nc.dram_tensor("masks", [128, 4 * SBK], f32, kind="ExternalInput").ap()
    z_d = nc.dram_tensor("z", [S, EMBED], f32, kind="ExternalOutput").ap()

    XG = 8           # x tiles per s-block (2 e-chunks each)
    XW = 2 * SBK     # x tile width

    with tile.TileContext(nc, pool_alloc_mode="queue") as tc, \
         nc.allow_low_precision(reason="fp32r attention pipeline"):
        pp = tc.alloc_tile_pool(name="persist", bufs=1)
        ps = tc.alloc_tile_pool(name="ps", bufs=8, space="PSUM")
        qt = pp.tile([128, HPC * S], f32r, tag="qt")   # Q^T rope, per head
        kt = pp.tile([128, HPC * S], f32r, tag="kt")   # K^T rope, per head
        # x tiles shared by phases A and B
        xa = tc.alloc_tile_pool(name="xa", bufs=XG)

        # ---------------- Phase A: Q/K projections + RoPE ----------------
        # Per s-block: load 8 x-tiles (held in SBUF), sweep Q over all
        # e-chunks, then sweep K reusing the same x-tiles.  Q's RoPE
        # (DVE) hides under the K sweep and vice versa.
        wp = tc.alloc_tile_pool(name="wqk", bufs=1)
        cs = tc.alloc_tile_pool(name="cossin", bufs=2)
        rp = tc.alloc_tile_pool(name="ropetmp", bufs=1)
        wq_sb = wp.tile([128, NEC * CW], f32r, tag="wq")
        wk_sb = wp.tile([128, NEC * CW], f32r, tag="wk")

        def rope(psrc, dst, cos_sb, sin_sb, sb, h, nm):
            t1 = rp.tile([128, SBK], f32, tag="t1", name=f"t1{nm}{sb}_{h}")
            t2 = rp.tile([128, SBK], f32, tag="t2", name=f"t2{nm}{sb}_{h}")
            nc.vector.tensor_mul(t1[:], psrc[:], cos_sb[:])
            nc.vector.tensor_mul(t2[0:64, :], psrc[64:128, :], sin_sb[0:64, :])
            nc.vector.tensor_mul(t2[64:128, :], psrc[0:64, :], sin_sb[64:128, :])
            ds = slice(h * S + sb * SBK, h * S + (sb + 1) * SBK)
            nc.vector.tensor_add(dst[:, ds], t1[:], t2[:])

        for sb in range(NSB):
            ss = slice(sb * SBK, (sb + 1) * SBK)
            xts = []
            for g in range(XG):
                xt_g = xa.tile([128, XW], f32r, tag="x", name=f"x{sb}_{g}")
                src_ap = xt_d[g * 256:(g + 1) * 256, ss]
                nc.sync.dma_start(
                    xt_g[:].rearrange("p (c s) -> p c s", s=SBK),
                    src_ap.rearrange("(c p) s -> p c s", p=128).bitcast(f32r))
                xts.append(xt_g)
                if sb == 0:
                    # interleave weight loads so the first matmul starts early
                    nc.sync.dma_start(
                        wq_sb[:, g * 2 * CW:(g + 1) * 2 * CW].rearrange(
                            "p (c m) -> p c m", m=CW),
                        wq_d[g * 256:(g + 1) * 256, :].rearrange(
                            "(c p) m -> p c m", p=128).bitcast(f32r))
            # -------- Q sweep --------
            qp = [ps.tile([128, SBK], f32, tag="ps", name=f"qp{sb}_{_h}")
                  for _h in range(HPC)]
            for g in range(XG):
                for el in range(2):
                    ec = 2 * g + el
                    st, sp = (ec == 0), (ec == NEC - 1)
                    xv = xts[g][:, el * SBK:(el + 1) * SBK]
                    for h in range(HPC):
                        wsl = slice(ec * CW + h * HD, ec * CW + (h + 1) * HD)
                        nc.tensor.matmul(qp[h][:], lhsT=wq_sb[:, wsl],
                                         rhs=xv, start=st, stop=sp)
            cos_sb = cs.tile([128, SBK], f32, tag="cos", name=f"cos{sb}")
            sin_sb = cs.tile([128, SBK], f32, tag="sin", name=f"sin{sb}")
            nc.sync.dma_start(cos_sb[:], cos_d[:, ss])
            nc.sync.dma_start(sin_sb[:], sin_d[:, ss])
            for h in range(HPC):
                rope(qp[h], qt, cos_sb, sin_sb, sb, h, "q")
            # -------- K sweep --------
            kp = [ps.tile([128, SBK], f32, tag="ps", name=f"kp{sb}_{_h}")
                  for _h in range(HPC)]
            for g in range(XG):
                if sb == 0:
                    nc.sync.dma_start(
                        wk_sb[:, g * 2 * CW:(g + 1) * 2 * CW].rearrange(
                            "p (c m) -> p c m", m=CW),
                        wk_d[g * 256:(g + 1) * 256, :].rearrange(
                            "(c p) m -> p c m", p=128).bitcast(f32r))
                for el in range(2):
                    ec = 2 * g + el
                    st, sp = (ec == 0), (ec == NEC - 1)
                    xv = xts[g][:, el * SBK:(el + 1) * SBK]
                    for h in range(HPC):
                        wsl = slice(ec * CW + h * HD, ec * CW + (h + 1) * HD)
                        nc.tensor.matmul(kp[h][:], lhsT=wk_sb[:, wsl],
                                         rhs=xv, start=st, stop=sp)
            for h in range(HPC):
                rope(kp[h], kt, cos_sb, sin_sb, sb, h, "k")

        rp.release()
        cs.release()
        wp.release()

        # vt + masks live through B and C
        vmp = tc.alloc_tile_pool(name="vtmsk", bufs=1, side="right")
        vt = vmp.tile([128, NST * CW], f32r, tag="vt")
        msk_sb = vmp.tile([128, 4 * SBK], f32r, tag="msk")
        nc.sync.dma_start(msk_sb[:], msk_d[:].bitcast(f32r))
        ones_col = msk_sb[:, 511:512]   # all-ones [128,1]
        ones_row = msk_sb[0:1, 0:128]   # all-ones [1,128]

        # ---------------- Phase B: V projection ----------------
        wvp = tc.alloc_tile_pool(name="wv", bufs=1, side="right")
        wv_sb = wvp.tile([128, NEC * CW], f32r, tag="wv")
        for sb in range(NSB):
            ss = slice(sb * SBK, (sb + 1) * SBK)
            vp = [ps.tile([128, CW], f32, tag="ps", name=f"vp{sb}_{_s}")
                  for _s in range(4)]
            xts = []
            for g in range(XG):
                xt_g = xa.tile([128, XW], f32r, tag="x", name=f"xb{sb}_{g}")
                src_ap = xt_d[g * 256:(g + 1) * 256, ss]
                nc.sync.dma_start(
                    xt_g[:].rearrange("p (c s) -> p c s", s=SBK),
                    src_ap.rearrange("(c p) s -> p c s", p=128).bitcast(f32r))
                xts.append(xt_g)
                if sb == 0:
                    nc.sync.dma_start(
                        wv_sb[:, g * 2 * CW:(g + 1) * 2 * CW].rearrange(
                            "p (c m) -> p c m", m=CW),
                        wv_d[g * 256:(g + 1) * 256, :].rearrange(
                            "(c p) m -> p c m", p=128).bitcast(f32r))
            for g in range(XG):
                for el in range(2):
                    ec = 2 * g + el
                    st, sp = (ec == 0), (ec == NEC - 1)
                    for sub in range(4):
                        nc.tensor.matmul(
                            vp[sub][:],
                            lhsT=xts[g][:, el * SBK + sub * 128:
                                        el * SBK + (sub + 1) * 128],
                            rhs=wv_sb[:, ec * CW:(ec + 1) * CW],
                            start=st, stop=sp)
            for sub in range(4):
                stile = sb * 4 + sub
                nc.scalar.copy(vt[:, stile * CW:(stile + 1) * CW], vp[sub][:])

        wvp.release()
        xa.release()

        # ---------------- Phase C: attention per head ----------------
        otp = tc.alloc_tile_pool(name="otp", bufs=1)
        ot = otp.tile([128, HPC * S], f32r, tag="ot")
        lap = tc.alloc_tile_pool(name="lall", bufs=1)
        l_all = lap.tile([1, 16 * SBK], f32, tag="lall")
        ptp = tc.alloc_tile_pool(name="pts", bufs=4)
        for h in range(HPC):
            for j in range(NSB):
                blk = h * NSB + j
                nkt = 4 * j + 4  # causal: k tiles 0..4j+3
                avp = ps.tile([128, SBK], f32, tag="ps", name=f"av{h}_{j}")
                lfull = ps.tile([128, SBK], f32, tag="ps", name=f"l{h}_{j}")
                lp = lfull[0:1, :]
                qs = slice(h * S + j * SBK, h * S + (j + 1) * SBK)
                for i in range(nkt):
                    sp_t = ps.tile([128, SBK], f32, tag="ps",
                                   name=f"s{h}_{j}_{i}")
                    ks = slice(h * S + i * 128, h * S + (i + 1) * 128)
                    nc.tensor.matmul(sp_t[:], lhsT=kt[:, ks], rhs=qt[:, qs],
                                     start=True, stop=True)
                    pt_sb = ptp.tile([128, SBK], f32r, tag="p",
                                     name=f"p{h}_{j}_{i}")
                    nc.scalar.activation(pt_sb[:], sp_t[:], EXP, scale=SCALE)
                    o_idx = i - 4 * j
                    if o_idx >= 0:  # diagonal-crossing tile
                        nc.vector.tensor_mul(
                            pt_sb[:], pt_sb[:],
                            msk_sb[:, o_idx * SBK:(o_idx + 1) * SBK])
                    st, sp = (i == 0), (i == nkt - 1)
                    nc.tensor.matmul(
                        avp[:],
                        lhsT=vt[:, i * CW + h * HD:i * CW + (h + 1) * HD],
                        rhs=pt_sb[:], start=st, stop=sp)
                    nc.tensor.matmul(lp, lhsT=ones_col, rhs=pt_sb[:],
                                     start=st, stop=sp)
                # stash unnormalized O^T and the row-sums
                nc.vector.tensor_copy(ot[:, qs], avp[:])
                nc.vector.tensor_copy(
                    l_all[0:1, blk * SBK:(blk + 1) * SBK], lp)
        ptp.release()

        # batched normalization: 1/l = exp(-ln l)
        lnp = tc.alloc_tile_pool(name="lnt", bufs=2)
        bcp = tc.alloc_tile_pool(name="bcs", bufs=2)
        for h in range(HPC):
            lnt = lnp.tile([1, NSB * SBK], f32r, tag="lnt", name=f"lnt{h}")
            hs = slice(h * NSB * SBK, (h + 1) * NSB * SBK)
            nc.scalar.activation(lnt[:], l_all[0:1, hs], LN)
            for j in range(NSB):
                bcps = ps.tile([128, SBK], f32, tag="ps", name=f"bc{h}_{j}")
                nc.tensor.matmul(bcps[:], lhsT=ones_row,
                                 rhs=lnt[0:1, j * SBK:(j + 1) * SBK],
                                 start=True, stop=True)
                bc_sb = bcp.tile([128, SBK], f32, tag="bcs", name=f"bcs{h}_{j}")
                nc.scalar.activation(bc_sb[:], bcps[:], EXP, scale=-1.0)
                qs = slice(h * S + j * SBK, h * S + (j + 1) * SBK)
                nc.vector.tensor_mul(ot[:, qs], ot[:, qs], bc_sb[:])
        bcp.release()
        lnp.release()
        lap.release()
        vmp.release()

        # ---------------- Phase D: output projection ----------------
        wop = tc.alloc_tile_pool(name="wo", bufs=1)
        zp = tc.alloc_tile_pool(name="zsb", bufs=2)
        wo_sb = wop.tile([128, HPC * EMBED], f32r, tag="wo")
        for h in range(HPC):
            nc.sync.dma_start(
                wo_sb[:, h * EMBED:(h + 1) * EMBED],
                wo_d[h * 128:(h + 1) * 128, :].bitcast(f32r))
        for q_i in range(NST):
            z_sb = zp.tile([128, EMBED], f32, tag="zs", name=f"zs{q_i}")
            for eb in range(4):
                zps = ps.tile([128, SBK], f32, tag="ps", name=f"z{q_i}_{eb}")
                for h in range(HPC):
                    nc.tensor.matmul(
                        zps[:],
                        lhsT=ot[:, h * S + q_i * 128:h * S + (q_i + 1) * 128],
                        rhs=wo_sb[:, h * EMBED + eb * SBK:
                                  h * EMBED + (eb + 1) * SBK],
                        start=(h == 0), stop=(h == HPC - 1))
                nc.vector.tensor_copy(z_sb[:, eb * SBK:(eb + 1) * SBK], zps[:])
            nc.sync.dma_start(z_d[q_i * 128:(q_i + 1) * 128, :], z_sb[:])
        zp.release()
        wop.release()
        otp.release()
        pp.release()
        ps.release()

    nc.compile()
    return nc


def _host_tables():
    inv_freq = 1.0 / (ROPE_BASE ** (np.arange(0, HD, 2, dtype=np.float64) / HD))
    ang = np.arange(S, dtype=np.float64)[:, None] * inv_freq[None, :]  # [S, 64]
    cos = np.cos(ang)
    sin = np.sin(ang)
    cost = np.ascontiguousarray(
        np.concatenate([cos, cos], axis=1).T.astype(np.float32))  # [128, S]
    sints = np.ascontiguousarray(
        np.concatenate([-sin, sin], axis=1).T.astype(np.float32))
    kk = np.arange(128)[:, None]
    qq = np.arange(SBK)[None, :]
    masks = np.zeros((128, 4 * SBK), dtype=np.float32)
    for o in range(4):
        masks[:, o * SBK:(o + 1) * SBK] = (kk <= qq - o * 128).astype(np.float32)
    return cost, sints, masks


def _in_maps(x, Wq, Wk, Wv, Wo):
    cost, sints, masks = _host_tables()
    maps = []
    for c in range(N_CORES):
        b = c // 4
        h0 = (c % 4) * CW  # column offset of this core's 4 heads
        maps.append({
            "xt": np.ascontiguousarray(x[b].T),
            "wq": np.ascontiguousarray(Wq[:, h0:h0 + CW]),
            "wk": np.ascontiguousarray(Wk[:, h0:h0 + CW]),
            "wv": np.ascontiguousarray(Wv[:, h0:h0 + CW]),
            "wo": np.ascontiguousarray(Wo[h0:h0 + CW, :]),
            "cost": cost,
            "sints": sints,
            "masks": masks,
        })
    return maps


def kernel(x, Wq, Wk, Wv, Wo):
    from concourse.bass_utils import run_bass_kernel_spmd

    x = np.asarray(x, dtype=np.float32)
    Wq = np.asarray(Wq, dtype=np.float32)
    Wk = np.asarray(Wk, dtype=np.float32)
    Wv = np.asarray(Wv, dtype=np.float32)
    Wo = np.asarray(Wo, dtype=np.float32)

    if "nc" not in _CACHE:
        _CACHE["nc"] = _build_program()
    nc = _CACHE["nc"]

    res = run_bass_kernel_spmd(nc, _in_maps(x, Wq, Wk, Wv, Wo),
                               core_ids=list(range(N_CORES)))
    zs = [res.results[c]["z"] for c in range(N_CORES)]
    out = np.empty((B, S, EMBED), dtype=np.float32)
    out[0] = zs[0] + zs[1] + zs[2] + zs[3]
    out[1] = zs[4] + zs[5] + zs[6] + zs[7]
    return out
